# revision 1
# baseline (speedup 1.0000x reference)
"""Trainium2 Bass kernel for an AttentionBlock (LN -> QKV -> attn -> out-proj + residual).

Shapes (hardcoded per problem spec): B=8, L=1024, C=1024, H=8 heads.
The reference uses a raw row-major reshape (torch-style .view) of q/k/v from
[B, L, C] to [B*H, L, C/H]; with L=1024, C=1024, H=8 this makes each
"attention head" operate on a contiguous 128-sequence-row block of the
[L, C] matrix, reinterpreted as [1024, 128].

Sharding: pure data-parallel over batch, one batch element per NeuronCore
(8 cores). No collectives.
"""

import math
from contextlib import ExitStack

import ml_dtypes
import numpy as np

import concourse.bass as bass
import concourse.bacc as bacc
import concourse.tile as tile
from concourse import mybir
from concourse import bass_utils
from concourse.masks import make_identity

L = 1024
C = 1024
H = 8          # heads; also number of 128-row l-tiles (head h <-> l-tile h)
CH = 128       # head dim
NT = 8         # l tiles (128 rows each)
NG = 8         # c groups (128 cols each)
EPS = 1e-5
S2 = 1.0 / math.sqrt(CH)   # combined q&k scale: (ch^-0.25)^2

f32 = mybir.dt.float32
f32r = mybir.dt.float32r
bf16 = mybir.dt.bfloat16
AF = mybir.ActivationFunctionType
ALU = mybir.AluOpType



def _bcast_ap(ap, p=128):
    """Broadcast a 1-D DRAM vector across p partitions (step-0 partition dim)."""
    return bass.AP(tensor=ap.tensor, offset=ap.offset, ap=[[0, p]] + list(ap.ap))


def _emit(nc, apply_affine: bool):
    x_d = nc.dram_tensor("x", [L, C], f32, kind="ExternalInput").ap()
    wqkv_d = nc.dram_tensor("w_qkv", [C, 3 * C], f32r, kind="ExternalInput").ap()
    bqk_d = nc.dram_tensor("b_qk", [128, 16], f32, kind="ExternalInput").ap()
    wout_d = nc.dram_tensor("w_out", [C, C], bf16, kind="ExternalInput").ap()
    bout_d = nc.dram_tensor("b_out_eff", [C], f32, kind="ExternalInput").ap()
    if apply_affine:
        g_d = nc.dram_tensor("ln_g", [C], f32, kind="ExternalInput").ap()
        b_d = nc.dram_tensor("ln_b", [C], f32, kind="ExternalInput").ap()
    out_d = nc.dram_tensor("out", [L, C], f32, kind="ExternalOutput").ap()

    with nc.allow_low_precision(reason="bf16/f32r compute by design"), \
         tile.TileContext(nc) as tc, ExitStack() as ctx:
        # Long-lived pools on the LEFT side, allocation order chosen so that
        # the ones dying earliest are on top of the stack.
        const = ctx.enter_context(tc.tile_pool(name="const", bufs=1, side="left"))
        ident = const.tile([128, 128], f32)
        make_identity(nc, ident)
        ones_bf = const.tile([128, 1], bf16)
        nc.vector.memset(ones_bf, 1.0)
        eps_sb = const.tile([128, 1], f32)
        nc.vector.memset(eps_sb, EPS)
        bqk_sb = const.tile([128, 16], f32)
        nc.sync.dma_start(out=bqk_sb[:], in_=bqk_d)
        bv_d = nc.dram_tensor("b_v", [C], f32, kind="ExternalInput").ap()
        bv_bc = const.tile([128, C], f32)
        nc.gpsimd.dma_start(out=bv_bc[:], in_=_bcast_ap(bv_d))
        if apply_affine:
            g_bc = const.tile([128, C], f32)
            nc.gpsimd.dma_start(out=g_bc[:], in_=_bcast_ap(g_d))
            b_bc = const.tile([128, C], f32)
            nc.gpsimd.dma_start(out=b_bc[:], in_=_bcast_ap(b_d))

        xn_pool = ctx.enter_context(tc.tile_pool(name="xn", bufs=1, side="left"))
        xn = xn_pool.tile([128, NT, C], f32)     # normalized x, natural [l, c]
        attnT_pool = ctx.enter_context(tc.tile_pool(name="attnT", bufs=1, side="left"))
        attnT = attnT_pool.tile([128, NG, L], bf16)   # [c', g_q, l]
        v_pool = tc.alloc_tile_pool(name="v", bufs=1, side="left")
        v_bf = v_pool.tile([128, NT, C], bf16)   # [l_r, l-tile, c]
        wv_pool = tc.alloc_tile_pool(name="wv", bufs=1, side="left")
        wv_sb = wv_pool.tile([128, NG, C], f32r)

        # ---------------- Phase 1: LayerNorm ----------------
        with tc.tile_pool(name="xin", bufs=8, side="right") as xin, \
             tc.tile_pool(name="lnst", bufs=4, side="right") as lnst, \
             tc.tile_pool(name="lntmp", bufs=3, side="right") as lntmp:
            for t in range(NT):
                xt = xin.tile([128, C], f32)
                stats = lnst.tile([128, 2, 6], f32)
                for j in range(2):
                    nc.sync.dma_start(
                        out=xt[:, 512 * j:512 * (j + 1)],
                        in_=x_d[128 * t:128 * (t + 1), 512 * j:512 * (j + 1)])
                    nc.vector.bn_stats(out=stats[:, j, :],
                                       in_=xt[:, 512 * j:512 * (j + 1)])
                mv = lnst.tile([128, 2], f32)
                nc.vector.bn_aggr(out=mv[:], in_=stats[:])
                sq = lnst.tile([128, 1], f32)
                nc.scalar.activation(out=sq[:], in_=mv[:, 1:2], func=AF.Sqrt,
                                     bias=eps_sb[:], scale=1.0)
                rstd = lnst.tile([128, 1], f32)
                nc.vector.reciprocal(out=rstd[:], in_=sq[:])
                nmr = lnst.tile([128, 1], f32)
                nc.vector.tensor_scalar(nmr[:], mv[:, 0:1], rstd[:], -1.0,
                                        ALU.mult, ALU.mult)
                if apply_affine:
                    zt = lntmp.tile([128, C], f32)
                    nc.scalar.activation(out=zt[:], in_=xt[:], func=AF.Identity,
                                         bias=nmr[:], scale=rstd[:])
                    zg = lntmp.tile([128, C], f32)
                    nc.vector.tensor_tensor(out=zg[:], in0=zt[:], in1=g_bc[:],
                                            op=ALU.mult)
                    nc.vector.tensor_tensor(out=xn[:, t, :], in0=zg[:], in1=b_bc[:],
                                            op=ALU.add)
                else:
                    nc.scalar.activation(out=xn[:, t, :], in_=xt[:], func=AF.Identity,
                                         bias=nmr[:], scale=rstd[:])

        # ------- Phase 2-4: transpose xn -> xnT; V, Q, K projections -------
        nc.sync.dma_start(
            out=wv_sb[:],
            in_=wqkv_d[:, 2 * C:3 * C].rearrange("(k p) n -> p k n", p=128))
        with tc.tile_pool(name="xnT", bufs=1, side="right") as xnT_pool:
            xnT = xnT_pool.tile([128, NG, L], f32r)   # [c', g, l]
            with tc.tile_pool(name="tr_ps", bufs=6, space="PSUM") as tr_ps:
                for t in range(NT):
                    for g in range(NG):
                        ps = tr_ps.tile([128, 128], f32)
                        nc.tensor.transpose(ps[:], xn[:, t, 128 * g:128 * (g + 1)],
                                            ident[:])
                        dst = xnT[:, g, 128 * t:128 * (t + 1)]
                        if g % 2 == 0:
                            nc.scalar.copy(out=dst, in_=ps[:])
                        else:
                            nc.vector.tensor_copy(dst, ps[:])

            with tc.tile_pool(name="proj_ps", bufs=4, space="PSUM") as proj_ps:
                # V projection (natural layout, bf16 out)
                for m in range(NT):
                    psv = proj_ps.tile([128, C], f32, tag="proj")
                    for ki in range(NG):
                        lhsT = xnT[:, ki, 128 * m:128 * (m + 1)]
                        for j in range(2):
                            nc.tensor.matmul(
                                psv[:, 512 * j:512 * (j + 1)], lhsT,
                                wv_sb[:, ki, 512 * j:512 * (j + 1)],
                                start=(ki == 0), stop=(ki == NG - 1))
                    nc.vector.tensor_tensor(out=v_bf[:, m, :], in0=psv[:],
                                            in1=bv_bc[:], op=ALU.add)
                wv_pool.release()

                # Q, K projections (transposed layout)
                qT_pool = tc.alloc_tile_pool(name="qT", bufs=1, side="left")
                qT = qT_pool.tile([128, H, NG, 128], bf16)   # [c', h, g_q, l_r]
                kT_pool = tc.alloc_tile_pool(name="kT", bufs=1, side="left")
                kT = kT_pool.tile([128, NG, L], bf16)        # [c', g_k, l]
                with tc.tile_pool(name="wqk", bufs=8, side="right") as wqk_pool:
                    for co in range(16):
                        wslab = wqk_pool.tile([128, NG, 128], f32r)
                        nc.sync.dma_start(
                            out=wslab[:],
                            in_=wqkv_d[:, 128 * co:128 * (co + 1)].rearrange(
                                "(k p) n -> p k n", p=128))
                        psq = proj_ps.tile([128, L], f32, tag="proj")
                        for ki in range(NG):
                            for j in range(2):
                                nc.tensor.matmul(
                                    psq[:, 512 * j:512 * (j + 1)],
                                    wslab[:, ki, :],
                                    xnT[:, ki, 512 * j:512 * (j + 1)],
                                    start=(ki == 0), stop=(ki == NG - 1))
                        bias_col = bqk_sb[:, co:co + 1]
                        if co < 8:
                            # q: dst [c', h, l_r] over h (l = 128h + l_r)
                            nc.vector.tensor_scalar(
                                qT[:, :, co, :],
                                psq[:].rearrange("p (h l) -> p h l", h=H),
                                bias_col, None, ALU.add)
                        else:
                            nc.vector.tensor_scalar(kT[:, co - 8, :], psq[:],
                                                    bias_col, None, ALU.add)

        # ---------------- Phase 5: attention ----------------
        pt_bufs = 2 if apply_affine else 3
        wout_pool = tc.alloc_tile_pool(name="wout", bufs=1, side="right")
        wout_sb = wout_pool.tile([128, NG, C], bf16)
        nc.sync.dma_start(out=wout_sb[:],
                          in_=wout_d.rearrange("(k p) n -> p k n", p=128))
        with tc.tile_pool(name="pt", bufs=pt_bufs, side="right") as pt_pool, \
             tc.tile_pool(name="rb", bufs=3, side="right") as rb_pool, \
             tc.tile_pool(name="recip", bufs=3, side="right") as recip_pool, \
             tc.tile_pool(name="s_ps", bufs=2, space="PSUM", side="right") as s_ps, \
             tc.tile_pool(name="sum_ps", bufs=1, space="PSUM") as sum_ps, \
             tc.tile_pool(name="av_ps", bufs=1, space="PSUM") as av_ps:
            pend = []   # (h, pt, rb) awaiting attnV; emitted one head behind

            def emit_scores(h):
                pt = pt_pool.tile([128, NG, L], bf16, name=f"pt{h}", tag="pt")
                hs = slice(128 * h, 128 * (h + 1))
                ps_sum = sum_ps.tile([1, L], f32, tag="ps_sum")
                qrow = qT[:, h, :, :].rearrange("p g l -> p (g l)")

                def emit_sums(gk):
                    for j in range(2):
                        nc.tensor.matmul(ps_sum[:, 512 * j:512 * (j + 1)], ones_bf[:],
                                         pt[:, gk, 512 * j:512 * (j + 1)],
                                         start=(gk == 0), stop=(gk == NG - 1))

                for gk in range(NG):
                    ps_s = s_ps.tile([128, L], f32, tag="ps_s")
                    for j in range(2):
                        nc.tensor.matmul(ps_s[:, 512 * j:512 * (j + 1)],
                                         kT[:, gk, hs],
                                         qrow[:, 512 * j:512 * (j + 1)],
                                         start=True, stop=True)
                    nc.scalar.activation(out=pt[:, gk, :], in_=ps_s[:], func=AF.Exp,
                                         bias=0.0, scale=S2)
                    if gk > 0:
                        emit_sums(gk - 1)
                emit_sums(NG - 1)
                recip = recip_pool.tile([1, L], f32, tag="recip")
                nc.vector.reciprocal_approx_fast(out=recip[:], in_=ps_sum[:])
                rb = rb_pool.tile([128, L], f32, tag="rb")
                nc.gpsimd.partition_broadcast(rb[:], recip[:])
                pend.append((h, pt, rb))

            def emit_attnv():
                h, pt, rb = pend.pop(0)
                hs = slice(128 * h, 128 * (h + 1))
                ps_av = av_ps.tile([128, L], f32, tag="ps_av")
                for gk in range(NG):
                    for j in range(2):
                        nc.tensor.matmul(ps_av[:, 512 * j:512 * (j + 1)],
                                         v_bf[:, h, 128 * gk:128 * (gk + 1)],
                                         pt[:, gk, 512 * j:512 * (j + 1)],
                                         start=(gk == 0), stop=(gk == NG - 1))
                # attnT[:, g_q, 128h + l_r] = ps_av[:, (g_q, l_r)] * rb
                nc.vector.tensor_tensor(
                    out=attnT[:, :, hs],
                    in0=ps_av[:].rearrange("p (g l) -> p g l", g=NG),
                    in1=rb[:].rearrange("p (g l) -> p g l", g=NG), op=ALU.mult)

            for h in range(H):
                emit_scores(h)
                if pend and h > 0:
                    emit_attnv()
            while pend:
                emit_attnv()

        kT_pool.release()
        qT_pool.release()
        v_pool.release()

        # ---------------- Phase 6: output projection + residual ----------------
        with tc.tile_pool(name="otile", bufs=4, side="right") as ot_pool, \
             tc.tile_pool(name="out_ps", bufs=3, space="PSUM") as out_ps:
            bout_bc = ot_pool.tile([128, C], f32)
            nc.gpsimd.dma_start(out=bout_bc[:], in_=_bcast_ap(bout_d))
            for m in range(NT):
                ps_o = out_ps.tile([128, C], f32)
                for ki in range(NG):
                    lhsT = attnT[:, ki, 128 * m:128 * (m + 1)]
                    for j in range(2):
                        nc.tensor.matmul(
                            ps_o[:, 512 * j:512 * (j + 1)], lhsT,
                            wout_sb[:, ki, 512 * j:512 * (j + 1)],
                            start=(ki == 0), stop=(ki == NG - 1))
                t1 = ot_pool.tile([128, C], f32)
                nc.vector.tensor_tensor(out=t1[:], in0=ps_o[:], in1=xn[:, m, :],
                                        op=ALU.add)
                t2 = ot_pool.tile([128, C], f32)
                nc.vector.tensor_tensor(out=t2[:], in0=t1[:], in1=bout_bc[:],
                                        op=ALU.add)
                nc.sync.dma_start(out=out_d[128 * m:128 * (m + 1), :], in_=t2[:])

        wout_pool.release()

    return nc


_CACHE = {}


def _build(apply_affine: bool):
    key = apply_affine
    if key not in _CACHE:
        nc = bacc.Bacc("TRN2", target_bir_lowering=False, debug=False)
        _emit(nc, apply_affine)
        nc.compile()
        _CACHE[key] = nc
    return _CACHE[key]


def kernel(**inputs) -> np.ndarray:
    x = np.asarray(inputs["x"], np.float32)
    ln_g = np.asarray(inputs["ln_g"], np.float32)
    ln_b = np.asarray(inputs["ln_b"], np.float32)
    w_qkv = np.ascontiguousarray(np.asarray(inputs["w_qkv"], np.float32))
    b_qkv = np.asarray(inputs["b_qkv"], np.float32)
    w_out = np.ascontiguousarray(np.asarray(inputs["w_out"], np.float32))
    b_out = np.asarray(inputs["b_out"], np.float32)

    B = x.shape[0]
    assert x.shape == (B, L, C)
    apply_affine = not (np.all(ln_g == 1.0) and np.all(ln_b == 0.0))
    nc = _build(apply_affine)

    b_out_eff = b_out
    bqk_pre = np.ascontiguousarray(b_qkv[:2 * C].reshape(16, 128).T)
    bv = np.ascontiguousarray(b_qkv[2 * C:])
    w_out_bf = w_out.astype(ml_dtypes.bfloat16)

    in_maps = []
    for c in range(B):
        m = {
            "x": np.ascontiguousarray(x[c]),
            "w_qkv": w_qkv,
            "b_qk": bqk_pre,
            "b_v": bv,
            "w_out": w_out_bf,
            "b_out_eff": b_out_eff,
        }
        if apply_affine:
            m["ln_g"] = ln_g
            m["ln_b"] = ln_b
        in_maps.append(m)

    res = bass_utils.run_bass_kernel_spmd(nc, in_maps, core_ids=list(range(B)))
    return np.stack([res.results[c]["out"] for c in range(B)]).astype(np.float32)



# revision 3
# speedup vs baseline: 1.5056x; 1.5056x over previous
"""Trainium2 Bass kernel for an AttentionBlock (LN -> QKV -> attn -> out-proj + residual).

Shapes (hardcoded per problem spec): B=8, L=1024, C=1024, H=8 heads.
The reference uses a raw row-major reshape (torch-style .view) of q/k/v from
[B, L, C] to [B*H, L, C/H]; with L=1024, C=1024, H=8 each "attention head"
operates on the [128, 1024] row-tile h of the [L, C] matrix: head h's
(l_q, c) element lives at xn[128h + l_q//8, 128*(l_q%8) + c].

Sharding: pure data-parallel over batch, one batch element per NeuronCore
(8 cores). No collectives.

Numerics: all large matmuls run in fp8 (e4m3, TRN max-normal 240), with
DoubleRow perf mode (contraction 256 per pass) for every matmul whose
contraction dim is 1024 (QKV proj, attn@V, softmax-sum, out-proj). The
scores matmul (contraction = head dim = 128) runs plain fp8. Weights are
pre-scaled by ALPHA=32 on the host so fp8 sees well-ranged values; the
combined ALPHA^2 factor is removed in the final residual add (and the
softmax exp scale absorbs it for scores). b_out rides as an extra fp8
contraction-pair in the out-projection matmul.
"""

import math
from contextlib import ExitStack

import ml_dtypes
import numpy as np

import concourse.bass as bass
import concourse.bacc as bacc
import concourse.tile as tile
from concourse import mybir
from concourse import bass_utils
from concourse.masks import make_identity

L = 1024
C = 1024
H = 8          # heads; also number of 128-row l-tiles (head h <-> l-tile h)
CH = 128       # head dim
NT = 8         # l tiles (128 rows each)
NG = 8         # c groups (128 cols each)
EPS = 1e-5
ALPHA = 32.0                   # host pre-scale on w_qkv/w_out (fp8 ranging)
DESCALE = 1.0 / (ALPHA * ALPHA)
S2 = 1.0 / math.sqrt(CH)       # combined q&k scale: (ch^-0.25)^2
S2A = S2 * DESCALE             # exp scale with the ALPHA^2 removed

f32 = mybir.dt.float32
bf16 = mybir.dt.bfloat16
f8 = mybir.dt.float8e4
AF = mybir.ActivationFunctionType
ALU = mybir.AluOpType
DR = mybir.MatmulPerfMode.DoubleRow


def _bcast_ap(ap, p=128):
    """Broadcast a 1-D DRAM vector across p partitions (step-0 partition dim)."""
    return bass.AP(tensor=ap.tensor, offset=ap.offset, ap=[[0, p]] + list(ap.ap))


def _emit(nc, apply_affine: bool):
    x_d = nc.dram_tensor("x", [L, C], f32, kind="ExternalInput").ap()
    wqk_d = nc.dram_tensor("w_qk8", [16, 128, NG, 128], f8, kind="ExternalInput").ap()
    wv_d = nc.dram_tensor("w_v8", [128, NG, C], f8, kind="ExternalInput").ap()
    wout_d = nc.dram_tensor("w_out8", [128, 10, C], f8, kind="ExternalInput").ap()
    bqk_d = nc.dram_tensor("b_qk", [128, 16], f32, kind="ExternalInput").ap()
    bv_d = nc.dram_tensor("b_v", [C], f32, kind="ExternalInput").ap()
    if apply_affine:
        g_d = nc.dram_tensor("ln_g", [C], f32, kind="ExternalInput").ap()
        b_d = nc.dram_tensor("ln_b", [C], f32, kind="ExternalInput").ap()
    out_d = nc.dram_tensor("out", [L, C], f32, kind="ExternalOutput").ap()

    with nc.allow_low_precision(reason="fp8 compute by design"), \
         tile.TileContext(nc) as tc, ExitStack() as ctx:
        # ------- long-lived left-side pools (release order is LIFO) -------
        const = ctx.enter_context(tc.tile_pool(name="const", bufs=1, side="left"))
        ident_bf = const.tile([128, 128], bf16)
        make_identity(nc, ident_bf)
        ones8 = const.tile([128, 2, 128], f8)
        nc.vector.memset(ones8, 1.0)
        eps_sb = const.tile([128, 1], f32)
        nc.vector.memset(eps_sb, EPS)
        bqk_sb = const.tile([128, 16], f32)
        nc.sync.dma_start(out=bqk_sb[:], in_=bqk_d)
        bv_bc = const.tile([128, C], f32)
        nc.gpsimd.dma_start(out=bv_bc[:], in_=_bcast_ap(bv_d))
        if apply_affine:
            g_bc = const.tile([128, C], f32)
            nc.gpsimd.dma_start(out=g_bc[:], in_=_bcast_ap(g_d))
            b_bc = const.tile([128, C], f32)
            nc.gpsimd.dma_start(out=b_bc[:], in_=_bcast_ap(b_d))

        xn_pool = ctx.enter_context(tc.tile_pool(name="xn", bufs=1, side="left"))
        xn_bf = xn_pool.tile([128, NT, C], bf16)     # normalized x, natural [l, c]
        attnT_pool = ctx.enter_context(tc.tile_pool(name="attnT", bufs=1, side="left"))
        attnT8 = attnT_pool.tile([128, 10, L], f8)   # [c', g_q, l]; rows 8,9 = bias
        nc.gpsimd.memset(attnT8[:, 8:10, :], 0.0)
        nc.gpsimd.memset(attnT8[0:1, 8, :], 1.0)
        v_pool = tc.alloc_tile_pool(name="v", bufs=1, side="left")
        v8 = v_pool.tile([128, NT, C], f8)           # [l_r, l-tile, c]
        qT_pool = tc.alloc_tile_pool(name="qT", bufs=1, side="left")
        qT8 = qT_pool.tile([128, H, NG, 128], f8)    # [c', h, g_q, l_r]
        kT_pool = tc.alloc_tile_pool(name="kT", bufs=1, side="left")
        kT8 = kT_pool.tile([128, NG, L], f8)         # [c', g_k, l]
        xnT_pool = tc.alloc_tile_pool(name="xnT", bufs=1, side="left")
        xnT8 = xnT_pool.tile([128, NG, L], f8)       # [c', g, l]
        wv_pool = tc.alloc_tile_pool(name="wv", bufs=1, side="left")
        wv8 = wv_pool.tile([128, NG, C], f8)
        nc.sync.dma_start(out=wv8[:], in_=wv_d)

        wout_pool = tc.alloc_tile_pool(name="wout", bufs=1, side="right")
        wout8 = wout_pool.tile([128, 10, C], f8)
        nc.sync.dma_start(out=wout8[:], in_=wout_d)

        # ---------------- Phase 1: LayerNorm ----------------
        with tc.tile_pool(name="xin", bufs=8, side="right") as xin, \
             tc.tile_pool(name="lnst", bufs=4, side="right") as lnst, \
             tc.tile_pool(name="lntmp", bufs=3, side="right") as lntmp:
            for t in range(NT):
                xt = xin.tile([128, C], f32)
                stats = lnst.tile([128, 2, 6], f32)
                for j in range(2):
                    nc.sync.dma_start(
                        out=xt[:, 512 * j:512 * (j + 1)],
                        in_=x_d[128 * t:128 * (t + 1), 512 * j:512 * (j + 1)])
                    nc.vector.bn_stats(out=stats[:, j, :],
                                       in_=xt[:, 512 * j:512 * (j + 1)])
                mv = lnst.tile([128, 2], f32)
                nc.vector.bn_aggr(out=mv[:], in_=stats[:])
                sq = lnst.tile([128, 1], f32)
                nc.scalar.activation(out=sq[:], in_=mv[:, 1:2], func=AF.Sqrt,
                                     bias=eps_sb[:], scale=1.0)
                rstd = lnst.tile([128, 1], f32)
                nc.vector.reciprocal(out=rstd[:], in_=sq[:])
                nmr = lnst.tile([128, 1], f32)
                nc.vector.tensor_scalar(nmr[:], mv[:, 0:1], rstd[:], -1.0,
                                        ALU.mult, ALU.mult)
                if apply_affine:
                    zt = lntmp.tile([128, C], f32)
                    nc.scalar.activation(out=zt[:], in_=xt[:], func=AF.Identity,
                                         bias=nmr[:], scale=rstd[:])
                    zg = lntmp.tile([128, C], f32)
                    nc.vector.tensor_tensor(out=zg[:], in0=zt[:], in1=g_bc[:],
                                            op=ALU.mult)
                    nc.vector.tensor_tensor(out=xn_bf[:, t, :], in0=zg[:],
                                            in1=b_bc[:], op=ALU.add)
                else:
                    nc.scalar.activation(out=xn_bf[:, t, :], in_=xt[:],
                                         func=AF.Identity, bias=nmr[:],
                                         scale=rstd[:])

        # ------- Phase 2: transpose xn (bf16) -> xnT8 (fp8) -------
        with tc.tile_pool(name="tr_ps", bufs=6, space="PSUM") as tr_ps:
            for t in range(NT):
                for g in range(NG):
                    ps = tr_ps.tile([128, 128], bf16)
                    nc.tensor.transpose(ps[:], xn_bf[:, t, 128 * g:128 * (g + 1)],
                                        ident_bf[:])
                    dst = xnT8[:, g, 128 * t:128 * (t + 1)]
                    if g % 2 == 0:
                        nc.scalar.copy(out=dst, in_=ps[:])
                    else:
                        nc.vector.tensor_copy(dst, ps[:])

        # ------- Phase 3-4: Q,K then V projections (all fp8 DoubleRow) -------
        pt_pool = tc.alloc_tile_pool(name="pt", bufs=3, side="right")
        rb_pool = tc.alloc_tile_pool(name="rb", bufs=2, side="right")
        pend = []   # (h, pt, rb) awaiting attnV; emitted one head behind

        proj_ps = tc.alloc_tile_pool(name="proj_ps", bufs=4, space="PSUM")

        # Q, K projections: psq[wcol, l] = sum_c' w[c', wcol] * xnT[c', l]
        with tc.tile_pool(name="wqk", bufs=8, side="right") as wqk_pool:
            for co in range(16):
                wslab = wqk_pool.tile([128, NG, 128], f8)
                nc.sync.dma_start(out=wslab[:], in_=wqk_d[co])
                psq = proj_ps.tile([128, L], f32, tag="proj")
                for kp in range(0, NG, 2):
                    for j in range(2):
                        nc.tensor.matmul(
                            psq[:, 512 * j:512 * (j + 1)],
                            wslab[:, kp:kp + 2, :],
                            xnT8[:, kp:kp + 2, 512 * j:512 * (j + 1)],
                            start=(kp == 0), stop=(kp == NG - 2),
                            perf_mode=DR)
                bias_col = bqk_sb[:, co:co + 1]
                if co < 8:
                    # q: dst [c', h, l_r] over h (l = 128h + l_r)
                    nc.vector.tensor_scalar(
                        qT8[:, :, co, :],
                        psq[:].rearrange("p (h l) -> p h l", h=H),
                        bias_col, None, ALU.add)
                else:
                    nc.vector.tensor_scalar(kT8[:, co - 8, :], psq[:],
                                            bias_col, None, ALU.add)

        # ---------------- Phase 5: attention ----------------
        def emit_scores(h, ps_pool, tag):
            pt = pt_pool.tile([128, NG, L], f8, name=f"pt{h}", tag="pt")
            hs = slice(128 * h, 128 * (h + 1))
            qrow = qT8[:, h, :, :].rearrange("p g l -> p (g l)")
            for gk in range(NG):
                ps_s = ps_pool.tile([128, L], f32, tag=tag)
                for j in range(2):
                    nc.tensor.matmul(ps_s[:, 512 * j:512 * (j + 1)],
                                     kT8[:, gk, hs],
                                     qrow[:, 512 * j:512 * (j + 1)],
                                     start=True, stop=True)
                nc.scalar.activation(out=pt[:, gk, :], in_=ps_s[:], func=AF.Exp,
                                     bias=0.0, scale=S2A)
            pend.append((h, pt))

        # head 0 scores share the projection PSUM pool (bank budget)
        emit_scores(0, proj_ps, "proj")

        # V projection (natural layout): psv[l, c] = sum_c' xnT[c', l] wv[c', c]
        for m in range(NT):
            psv = proj_ps.tile([128, C], f32, tag="proj")
            for kp in range(0, NG, 2):
                for j in range(2):
                    nc.tensor.matmul(
                        psv[:, 512 * j:512 * (j + 1)],
                        xnT8[:, kp:kp + 2, 128 * m:128 * (m + 1)],
                        wv8[:, kp:kp + 2, 512 * j:512 * (j + 1)],
                        start=(kp == 0), stop=(kp == NG - 2),
                        perf_mode=DR)
            nc.vector.tensor_tensor(out=v8[:, m, :], in0=psv[:], in1=bv_bc[:],
                                    op=ALU.add)
        wv_pool.release()
        xnT_pool.release()
        proj_ps.release()

        s_ps = tc.alloc_tile_pool(name="s_ps", bufs=2, space="PSUM")
        sum_ps = tc.alloc_tile_pool(name="sum_ps", bufs=1, space="PSUM")
        av_ps = tc.alloc_tile_pool(name="av_ps", bufs=1, space="PSUM")

        def emit_sums(h, pt):
            # denominator rows, broadcast to all 128 partitions by the ones lhsT
            ps_sum = sum_ps.tile([128, L], f32, tag="ps_sum")
            for kp in range(0, NG, 2):
                for j in range(2):
                    nc.tensor.matmul(ps_sum[:, 512 * j:512 * (j + 1)],
                                     ones8[:, :, 0:128],
                                     pt[:, kp:kp + 2, 512 * j:512 * (j + 1)],
                                     start=(kp == 0), stop=(kp == NG - 2),
                                     perf_mode=DR)
            rb = rb_pool.tile([128, L], f32, tag="rb")
            nc.vector.reciprocal_approx_fast(out=rb[:], in_=ps_sum[:])
            return rb

        def emit_attnv(h, pt, rb):
            hs = slice(128 * h, 128 * (h + 1))
            ps_av = av_ps.tile([128, L], f32, tag="ps_av")
            vh = v8[:, h, :].rearrange("p (g c) -> p g c", g=NG)
            for kp in range(0, NG, 2):
                for j in range(2):
                    nc.tensor.matmul(ps_av[:, 512 * j:512 * (j + 1)],
                                     vh[:, kp:kp + 2, :],
                                     pt[:, kp:kp + 2, 512 * j:512 * (j + 1)],
                                     start=(kp == 0), stop=(kp == NG - 2),
                                     perf_mode=DR)
            # attnT[:, g_q, 128h + l_r] = ps_av[:, (g_q, l_r)] * rb
            nc.vector.tensor_tensor(
                out=attnT8[:, 0:NG, hs],
                in0=ps_av[:].rearrange("p (g l) -> p g l", g=NG),
                in1=rb[:].rearrange("p (g l) -> p g l", g=NG), op=ALU.mult)

        for h in range(1, H):
            emit_scores(h, s_ps, "ps_s")
            hprev, ptprev = pend.pop(0)
            rbprev = emit_sums(hprev, ptprev)
            emit_attnv(hprev, ptprev, rbprev)
        hlast, ptlast = pend.pop(0)
        rblast = emit_sums(hlast, ptlast)
        emit_attnv(hlast, ptlast, rblast)

        av_ps.release()
        sum_ps.release()
        s_ps.release()
        kT_pool.release()
        qT_pool.release()
        v_pool.release()
        rb_pool.release()
        pt_pool.release()

        # ------- Phase 6: output projection + bias (fp8 DR) + residual -------
        with tc.tile_pool(name="otile", bufs=4, side="right") as ot_pool, \
             tc.tile_pool(name="out_ps", bufs=3, space="PSUM") as out_ps:
            for m in range(NT):
                ps_o = out_ps.tile([128, C], f32)
                for kp in range(0, 10, 2):   # pair 8,9 adds alpha^2 * b_out
                    for j in range(2):
                        nc.tensor.matmul(
                            ps_o[:, 512 * j:512 * (j + 1)],
                            attnT8[:, kp:kp + 2, 128 * m:128 * (m + 1)],
                            wout8[:, kp:kp + 2, 512 * j:512 * (j + 1)],
                            start=(kp == 0), stop=(kp == 8),
                            perf_mode=DR)
                t1 = ot_pool.tile([128, C], f32)
                nc.scalar.activation(out=t1[:], in_=ps_o[:], func=AF.Identity,
                                     bias=0.0, scale=DESCALE)
                t2 = ot_pool.tile([128, C], f32)
                nc.vector.tensor_tensor(out=t2[:], in0=t1[:], in1=xn_bf[:, m, :],
                                        op=ALU.add)
                nc.sync.dma_start(out=out_d[128 * m:128 * (m + 1), :], in_=t2[:])

        wout_pool.release()

    return nc


_CACHE = {}


def _build(apply_affine: bool):
    key = apply_affine
    if key not in _CACHE:
        nc = bacc.Bacc("TRN2", target_bir_lowering=False, debug=False)
        _emit(nc, apply_affine)
        nc.compile()
        _CACHE[key] = nc
    return _CACHE[key]


E4 = ml_dtypes.float8_e4m3


def prepare_weights(w_qkv, b_qkv, w_out, b_out):
    """Host-side fp8 packing/pre-scaling shared by kernel() and test.py."""
    w_qkv = np.ascontiguousarray(np.asarray(w_qkv, np.float32))
    b_qkv = np.asarray(b_qkv, np.float32)
    w_out = np.ascontiguousarray(np.asarray(w_out, np.float32))
    b_out = np.asarray(b_out, np.float32)

    # [co, p, k, n] = ALPHA * w_qkv[128k+p, 128co+n], co<16 (Q then K)
    wqk8 = np.ascontiguousarray(
        (ALPHA * w_qkv[:, :2 * C]).reshape(NG, 128, 16, 128)
        .transpose(2, 1, 0, 3)).astype(E4)
    # [p, k, n] = ALPHA * w_qkv[128k+p, 2048+n]
    wv8 = np.ascontiguousarray(
        (ALPHA * w_qkv[:, 2 * C:]).reshape(NG, 128, C)
        .transpose(1, 0, 2)).astype(E4)
    # [p, ki, n]: ki<8 weights; ki=8 partition0 = ALPHA^2*b_out; ki=9 zero
    wout8 = np.zeros((128, 10, C), np.float32)
    wout8[:, :NG, :] = (ALPHA * w_out).reshape(NG, 128, C).transpose(1, 0, 2)
    wout8[0, 8, :] = ALPHA * ALPHA * b_out
    wout8 = wout8.astype(E4)
    bqk = np.ascontiguousarray((ALPHA * b_qkv[:2 * C]).reshape(16, 128).T)
    bv = np.ascontiguousarray(ALPHA * b_qkv[2 * C:])
    return {"w_qk8": wqk8, "w_v8": wv8, "w_out8": wout8, "b_qk": bqk, "b_v": bv}


def kernel(**inputs) -> np.ndarray:
    x = np.asarray(inputs["x"], np.float32)
    ln_g = np.asarray(inputs["ln_g"], np.float32)
    ln_b = np.asarray(inputs["ln_b"], np.float32)

    B = x.shape[0]
    assert x.shape == (B, L, C)
    apply_affine = not (np.all(ln_g == 1.0) and np.all(ln_b == 0.0))
    nc = _build(apply_affine)

    w = prepare_weights(inputs["w_qkv"], inputs["b_qkv"],
                        inputs["w_out"], inputs["b_out"])

    in_maps = []
    for c in range(B):
        m = {"x": np.ascontiguousarray(x[c])}
        m.update(w)
        if apply_affine:
            m["ln_g"] = ln_g
            m["ln_b"] = ln_b
        in_maps.append(m)

    res = bass_utils.run_bass_kernel_spmd(nc, in_maps, core_ids=list(range(B)))
    return np.stack([res.results[c]["out"] for c in range(B)]).astype(np.float32)


# revision 6
# speedup vs baseline: 1.5527x; 1.0313x over previous
"""Trainium2 Bass kernel for an AttentionBlock (LN -> QKV -> attn -> out-proj + residual).

Shapes (hardcoded per problem spec): B=8, L=1024, C=1024, H=8 heads.
The reference uses a raw row-major reshape (torch-style .view) of q/k/v from
[B, L, C] to [B*H, L, C/H]; with L=1024, C=1024, H=8 each "attention head"
operates on the [128, 1024] row-tile h of the [L, C] matrix: head h's
(l_q, c) element lives at xn[128h + l_q//8, 128*(l_q%8) + c].

Sharding: pure data-parallel over batch, one batch element per NeuronCore
(8 cores). No collectives.

Numerics: all large matmuls run in fp8 (e4m3, TRN max-normal 240), with
DoubleRow perf mode (contraction 256 per pass) for every matmul whose
contraction dim is 1024 (QKV proj, attn@V, softmax-sum, out-proj). The
scores matmul (contraction = head dim = 128) runs plain fp8. Weights are
pre-scaled by ALPHA=32 on the host so fp8 sees well-ranged values; the
combined ALPHA^2 factor is removed in the final residual add (and the
softmax exp scale absorbs it for scores). b_out rides as an extra fp8
contraction-pair in the out-projection matmul.
"""

import math
from contextlib import ExitStack

import ml_dtypes
import numpy as np

import concourse.bass as bass
import concourse.bacc as bacc
import concourse.tile as tile
from concourse import mybir
from concourse import bass_utils
from concourse.masks import make_identity

L = 1024
C = 1024
H = 8          # heads; also number of 128-row l-tiles (head h <-> l-tile h)
CH = 128       # head dim
NT = 8         # l tiles (128 rows each)
NG = 8         # c groups (128 cols each)
EPS = 1e-5
ALPHA = 32.0                   # host pre-scale on w_qkv/w_out (fp8 ranging)
DESCALE = 1.0 / (ALPHA * ALPHA)
S2 = 1.0 / math.sqrt(CH)       # combined q&k scale: (ch^-0.25)^2
S2A = S2 * DESCALE             # exp scale with the ALPHA^2 removed

f32 = mybir.dt.float32
bf16 = mybir.dt.bfloat16
f8 = mybir.dt.float8e4
AF = mybir.ActivationFunctionType
ALU = mybir.AluOpType
DR = mybir.MatmulPerfMode.DoubleRow


def _bcast_ap(ap, p=128):
    """Broadcast a 1-D DRAM vector across p partitions (step-0 partition dim)."""
    return bass.AP(tensor=ap.tensor, offset=ap.offset, ap=[[0, p]] + list(ap.ap))


def _emit(nc, apply_affine: bool):
    x_d = nc.dram_tensor("x", [L, C], f32, kind="ExternalInput").ap()
    wqk_d = nc.dram_tensor("w_qk8", [16, 128, NG, 128], f8, kind="ExternalInput").ap()
    wv_d = nc.dram_tensor("w_v8", [128, NG, C], f8, kind="ExternalInput").ap()
    wout_d = nc.dram_tensor("w_out8", [128, 10, C], f8, kind="ExternalInput").ap()
    bqk_d = nc.dram_tensor("b_qk", [128, 16], f32, kind="ExternalInput").ap()
    bv_d = nc.dram_tensor("b_v", [C], f32, kind="ExternalInput").ap()
    if apply_affine:
        g_d = nc.dram_tensor("ln_g", [C], f32, kind="ExternalInput").ap()
        b_d = nc.dram_tensor("ln_b", [C], f32, kind="ExternalInput").ap()
    out_d = nc.dram_tensor("out", [L, C], f32, kind="ExternalOutput").ap()

    with nc.allow_low_precision(reason="fp8 compute by design"), \
         tile.TileContext(nc) as tc, ExitStack() as ctx:
        # ------- long-lived left-side pools (release order is LIFO) -------
        const = ctx.enter_context(tc.tile_pool(name="const", bufs=1, side="left"))
        ident_bf = const.tile([128, 128], bf16)
        make_identity(nc, ident_bf)   # first: gates the transposes
        ones8 = const.tile([128, 2, 128], f8)
        nc.vector.memset(ones8, 1.0)
        eps_sb = const.tile([128, 1], f32)
        nc.vector.memset(eps_sb, EPS)
        bqk_sb = const.tile([128, 16], f32)
        nc.sync.dma_start(out=bqk_sb[:], in_=bqk_d)
        bv_bc = const.tile([128, C], f32)
        nc.gpsimd.dma_start(out=bv_bc[:], in_=_bcast_ap(bv_d))
        if apply_affine:
            g_bc = const.tile([128, C], f32)
            nc.gpsimd.dma_start(out=g_bc[:], in_=_bcast_ap(g_d))
            b_bc = const.tile([128, C], f32)
            nc.gpsimd.dma_start(out=b_bc[:], in_=_bcast_ap(b_d))

        xn_pool = ctx.enter_context(tc.tile_pool(name="xn", bufs=1, side="left"))
        xn_bf = xn_pool.tile([128, NT, C], bf16)     # normalized x, natural [l, c]
        attnT_pool = ctx.enter_context(tc.tile_pool(name="attnT", bufs=1, side="left"))
        attnT8 = attnT_pool.tile([128, 10, L], f8)   # [c', g_q, l]; rows 8,9 = bias
        nc.gpsimd.memset(attnT8[:, 8:10, :], 0.0)
        nc.gpsimd.memset(attnT8[0:1, 8, :], 1.0)
        v_pool = tc.alloc_tile_pool(name="v", bufs=1, side="left")
        v8 = v_pool.tile([128, NT, C], f8)           # [l_r, l-tile, c]
        qT_pool = tc.alloc_tile_pool(name="qT", bufs=1, side="left")
        qT8 = qT_pool.tile([128, H, NG, 128], f8)    # [c', h, g_q, l_r]
        kT_pool = tc.alloc_tile_pool(name="kT", bufs=1, side="left")
        kT8 = kT_pool.tile([128, NG, L], f8)         # [c', g_k, l]
        xnT_pool = tc.alloc_tile_pool(name="xnT", bufs=1, side="left")
        xnT8 = xnT_pool.tile([128, NG, L], f8)       # [c', g, l]
        wv_pool = tc.alloc_tile_pool(name="wv", bufs=1, side="left")
        wv8 = wv_pool.tile([128, NG, C], f8)

        wout_pool = tc.alloc_tile_pool(name="wout", bufs=1, side="right")
        wout8 = wout_pool.tile([128, 10, C], f8)
        # NOTE: wv8/wout8 DMAs are emitted after the QK weight slabs so the
        # x-input DMAs (which gate LayerNorm -> transposes -> everything)
        # run first on the sync queue.

        # ---------------- Phase 1: LayerNorm ----------------
        with tc.tile_pool(name="xin", bufs=8, side="right") as xin, \
             tc.tile_pool(name="lnst", bufs=4, side="right") as lnst, \
             tc.tile_pool(name="lntmp", bufs=3, side="right") as lntmp:
            for t in range(NT):
                xt = xin.tile([128, C], f32)
                stats = lnst.tile([128, 2, 6], f32)
                for j in range(2):
                    nc.sync.dma_start(
                        out=xt[:, 512 * j:512 * (j + 1)],
                        in_=x_d[128 * t:128 * (t + 1), 512 * j:512 * (j + 1)])
                    nc.vector.bn_stats(out=stats[:, j, :],
                                       in_=xt[:, 512 * j:512 * (j + 1)])
                mv = lnst.tile([128, 2], f32)
                nc.vector.bn_aggr(out=mv[:], in_=stats[:])
                sq = lnst.tile([128, 1], f32)
                nc.scalar.activation(out=sq[:], in_=mv[:, 1:2], func=AF.Sqrt,
                                     bias=eps_sb[:], scale=1.0)
                rstd = lnst.tile([128, 1], f32)
                nc.vector.reciprocal(out=rstd[:], in_=sq[:])
                nmr = lnst.tile([128, 1], f32)
                nc.vector.tensor_scalar(nmr[:], mv[:, 0:1], rstd[:], -1.0,
                                        ALU.mult, ALU.mult)
                if apply_affine:
                    zt = lntmp.tile([128, C], f32)
                    nc.scalar.activation(out=zt[:], in_=xt[:], func=AF.Identity,
                                         bias=nmr[:], scale=rstd[:])
                    zg = lntmp.tile([128, C], f32)
                    nc.vector.tensor_tensor(out=zg[:], in0=zt[:], in1=g_bc[:],
                                            op=ALU.mult)
                    nc.vector.tensor_tensor(out=xn_bf[:, t, :], in0=zg[:],
                                            in1=b_bc[:], op=ALU.add)
                else:
                    nc.scalar.activation(out=xn_bf[:, t, :], in_=xt[:],
                                         func=AF.Identity, bias=nmr[:],
                                         scale=rstd[:])

        # ------- Phase 2: transpose xn (bf16) -> xnT8 (fp8) -------
        with tc.tile_pool(name="tr_ps", bufs=6, space="PSUM") as tr_ps:
            for t in range(NT):
                for g in range(NG):
                    ps = tr_ps.tile([128, 128], bf16)
                    nc.tensor.transpose(ps[:], xn_bf[:, t, 128 * g:128 * (g + 1)],
                                        ident_bf[:])
                    dst = xnT8[:, g, 128 * t:128 * (t + 1)]
                    if g % 2 == 0:
                        nc.scalar.copy(out=dst, in_=ps[:])
                    else:
                        nc.vector.tensor_copy(dst, ps[:])

        # ------- Phase 3-4: Q,K projections (fp8 DoubleRow) -------
        pt_pool = tc.alloc_tile_pool(name="pt", bufs=3, side="right")
        rb_pool = tc.alloc_tile_pool(name="rb", bufs=3, side="right")

        proj_ps = tc.alloc_tile_pool(name="proj_ps", bufs=4, space="PSUM")

        # Q, K projections: psq[wcol, l] = sum_c' w[c', wcol] * xnT[c', l]
        with tc.tile_pool(name="wqk", bufs=8, side="right") as wqk_pool:
            for co in range(16):
                wslab = wqk_pool.tile([128, NG, 128], f8)
                nc.sync.dma_start(out=wslab[:], in_=wqk_d[co])
                psq = proj_ps.tile([128, L], f32, tag="proj")
                for kp in range(0, NG, 2):
                    for j in range(2):
                        nc.tensor.matmul(
                            psq[:, 512 * j:512 * (j + 1)],
                            wslab[:, kp:kp + 2, :],
                            xnT8[:, kp:kp + 2, 512 * j:512 * (j + 1)],
                            start=(kp == 0), stop=(kp == NG - 2),
                            perf_mode=DR)
                bias_col = bqk_sb[:, co:co + 1]
                if co < 8:
                    # q: dst [c', h, l_r] over h (l = 128h + l_r)
                    nc.vector.tensor_scalar(
                        qT8[:, :, co, :],
                        psq[:].rearrange("p (h l) -> p h l", h=H),
                        bias_col, None, ALU.add)
                else:
                    nc.vector.tensor_scalar(kT8[:, co - 8, :], psq[:],
                                            bias_col, None, ALU.add)

        # weight DMAs for the later phases go on the queue only now, so they
        # don't delay the x/wqk DMAs above
        nc.sync.dma_start(out=wv8[:], in_=wv_d)
        nc.sync.dma_start(out=wout8[:], in_=wout_d)

        proj_ps.release()

        # ---------------- Phase 5: attention + V-projection ----------------
        s_ps = tc.alloc_tile_pool(name="s_ps", bufs=2, space="PSUM")
        sumav_ps = tc.alloc_tile_pool(name="sumav", bufs=2, space="PSUM")
        psv_ps = tc.alloc_tile_pool(name="psv", bufs=1, space="PSUM")

        def emit_scores(h):
            pt = pt_pool.tile([128, NG, L], f8, name=f"pt{h}", tag="pt")
            hs = slice(128 * h, 128 * (h + 1))
            qrow = qT8[:, h, :, :].rearrange("p g l -> p (g l)")
            for gk in range(NG):
                ps_s = s_ps.tile([128, L], f32, tag="ps_s")
                for j in range(2):
                    nc.tensor.matmul(ps_s[:, 512 * j:512 * (j + 1)],
                                     kT8[:, gk, hs],
                                     qrow[:, 512 * j:512 * (j + 1)],
                                     start=True, stop=True)
                nc.scalar.activation(out=pt[:, gk, :], in_=ps_s[:], func=AF.Exp,
                                     bias=0.0, scale=S2A)
            return pt

        def emit_vproj(m):
            # V projection (natural layout): psv[l, c] = sum_c' xnT[c',l] wv[c',c]
            psv = psv_ps.tile([128, C], f32, tag="psv")
            for kp in range(0, NG, 2):
                for j in range(2):
                    nc.tensor.matmul(
                        psv[:, 512 * j:512 * (j + 1)],
                        xnT8[:, kp:kp + 2, 128 * m:128 * (m + 1)],
                        wv8[:, kp:kp + 2, 512 * j:512 * (j + 1)],
                        start=(kp == 0), stop=(kp == NG - 2),
                        perf_mode=DR)
            nc.vector.tensor_tensor(out=v8[:, m, :], in0=psv[:], in1=bv_bc[:],
                                    op=ALU.add)

        def emit_sums(h, pt):
            # denominator rows, broadcast to all 128 partitions by the ones lhsT
            rb = rb_pool.tile([128, L], f32, tag="rb")
            for j in range(2):
                ps_sum = sumav_ps.tile([128, 512], f32, tag="sumav")
                for kp in range(0, NG, 2):
                    nc.tensor.matmul(ps_sum[:],
                                     ones8[:, :, 0:128],
                                     pt[:, kp:kp + 2, 512 * j:512 * (j + 1)],
                                     start=(kp == 0), stop=(kp == NG - 2),
                                     perf_mode=DR)
                nc.vector.reciprocal_approx_fast(
                    out=rb[:, 512 * j:512 * (j + 1)], in_=ps_sum[:])
            return rb

        def emit_attnv(h, pt, rb):
            hs = slice(128 * h, 128 * (h + 1))
            vh = v8[:, h, :].rearrange("p (g c) -> p g c", g=NG)
            for j in range(2):
                ps_av = sumav_ps.tile([128, 512], f32, tag="sumav")
                for kp in range(0, NG, 2):
                    nc.tensor.matmul(ps_av[:],
                                     vh[:, kp:kp + 2, :],
                                     pt[:, kp:kp + 2, 512 * j:512 * (j + 1)],
                                     start=(kp == 0), stop=(kp == NG - 2),
                                     perf_mode=DR)
                # attnT[:, g_q, 128h + l_r] = ps_av[:, (g_q, l_r)] * rb
                nc.vector.tensor_tensor(
                    out=attnT8[:, 4 * j:4 * (j + 1), hs],
                    in0=ps_av[:].rearrange("p (g l) -> p g l", g=4),
                    in1=rb[:, 512 * j:512 * (j + 1)].rearrange(
                        "p (g l) -> p g l", g=4),
                    op=ALU.mult)

        # software-pipelined head loop: scores h+1 / vproj h+1 / sums h /
        # attnV h-1 all in flight at once
        pts = {}
        rbs = {}
        pts[0] = emit_scores(0)
        emit_vproj(0)
        for h in range(H):
            if h + 1 < H:
                pts[h + 1] = emit_scores(h + 1)
                emit_vproj(h + 1)
            rbs[h] = emit_sums(h, pts[h])
            if h >= 1:
                emit_attnv(h - 1, pts[h - 1], rbs[h - 1])
                del pts[h - 1], rbs[h - 1]
        emit_attnv(H - 1, pts[H - 1], rbs[H - 1])

        psv_ps.release()
        sumav_ps.release()
        s_ps.release()
        wv_pool.release()
        xnT_pool.release()
        kT_pool.release()
        qT_pool.release()
        v_pool.release()
        rb_pool.release()
        pt_pool.release()

        # ------- Phase 6: output projection + bias (fp8 DR) + residual -------
        with tc.tile_pool(name="otile", bufs=4, side="right") as ot_pool, \
             tc.tile_pool(name="out_ps", bufs=3, space="PSUM") as out_ps:
            for m in range(NT):
                ps_o = out_ps.tile([128, C], f32)
                for kp in range(0, 10, 2):   # pair 8,9 adds alpha^2 * b_out
                    for j in range(2):
                        nc.tensor.matmul(
                            ps_o[:, 512 * j:512 * (j + 1)],
                            attnT8[:, kp:kp + 2, 128 * m:128 * (m + 1)],
                            wout8[:, kp:kp + 2, 512 * j:512 * (j + 1)],
                            start=(kp == 0), stop=(kp == 8),
                            perf_mode=DR)
                t1 = ot_pool.tile([128, C], f32)
                nc.scalar.activation(out=t1[:], in_=ps_o[:], func=AF.Identity,
                                     bias=0.0, scale=DESCALE)
                t2 = ot_pool.tile([128, C], f32)
                nc.vector.tensor_tensor(out=t2[:], in0=t1[:], in1=xn_bf[:, m, :],
                                        op=ALU.add)
                nc.sync.dma_start(out=out_d[128 * m:128 * (m + 1), :], in_=t2[:])

        wout_pool.release()

    return nc


_CACHE = {}


def _build(apply_affine: bool):
    key = apply_affine
    if key not in _CACHE:
        nc = bacc.Bacc("TRN2", target_bir_lowering=False, debug=False)
        _emit(nc, apply_affine)
        nc.compile()
        _CACHE[key] = nc
    return _CACHE[key]


E4 = ml_dtypes.float8_e4m3


def prepare_weights(w_qkv, b_qkv, w_out, b_out):
    """Host-side fp8 packing/pre-scaling shared by kernel() and test.py."""
    w_qkv = np.ascontiguousarray(np.asarray(w_qkv, np.float32))
    b_qkv = np.asarray(b_qkv, np.float32)
    w_out = np.ascontiguousarray(np.asarray(w_out, np.float32))
    b_out = np.asarray(b_out, np.float32)

    # [co, p, k, n] = ALPHA * w_qkv[128k+p, 128co+n], co<16 (Q then K)
    wqk8 = np.ascontiguousarray(
        (ALPHA * w_qkv[:, :2 * C]).reshape(NG, 128, 16, 128)
        .transpose(2, 1, 0, 3)).astype(E4)
    # [p, k, n] = ALPHA * w_qkv[128k+p, 2048+n]
    wv8 = np.ascontiguousarray(
        (ALPHA * w_qkv[:, 2 * C:]).reshape(NG, 128, C)
        .transpose(1, 0, 2)).astype(E4)
    # [p, ki, n]: ki<8 weights; ki=8 partition0 = ALPHA^2*b_out; ki=9 zero
    wout8 = np.zeros((128, 10, C), np.float32)
    wout8[:, :NG, :] = (ALPHA * w_out).reshape(NG, 128, C).transpose(1, 0, 2)
    wout8[0, 8, :] = ALPHA * ALPHA * b_out
    wout8 = wout8.astype(E4)
    bqk = np.ascontiguousarray((ALPHA * b_qkv[:2 * C]).reshape(16, 128).T)
    bv = np.ascontiguousarray(ALPHA * b_qkv[2 * C:])
    return {"w_qk8": wqk8, "w_v8": wv8, "w_out8": wout8, "b_qk": bqk, "b_v": bv}


def kernel(**inputs) -> np.ndarray:
    x = np.asarray(inputs["x"], np.float32)
    ln_g = np.asarray(inputs["ln_g"], np.float32)
    ln_b = np.asarray(inputs["ln_b"], np.float32)

    B = x.shape[0]
    assert x.shape == (B, L, C)
    apply_affine = not (np.all(ln_g == 1.0) and np.all(ln_b == 0.0))
    nc = _build(apply_affine)

    w = prepare_weights(inputs["w_qkv"], inputs["b_qkv"],
                        inputs["w_out"], inputs["b_out"])

    in_maps = []
    for c in range(B):
        m = {"x": np.ascontiguousarray(x[c])}
        m.update(w)
        if apply_affine:
            m["ln_g"] = ln_g
            m["ln_b"] = ln_b
        in_maps.append(m)

    res = bass_utils.run_bass_kernel_spmd(nc, in_maps, core_ids=list(range(B)))
    return np.stack([res.results[c]["out"] for c in range(B)]).astype(np.float32)
